# revision 17
# baseline (speedup 1.0000x reference)
"""Trainium2 Bass kernel for nn_Block_12154757448460 (spiking retention transformer).

Sharding over 8 NeuronCores (1 chip):
  - q,k projections: column-sharded by head (core h computes head h's 64 q-cols +
    64 k-cols for ALL rows) -> attention intra-chunk inputs stay local.
  - k,v projections (row-major copies), output projection, MLP: row-sharded
    (core c owns n in [128c, 128(c+1)) for all t,b).
  - attention + retention LIF: head-sharded (core h = head h, 16 (t,b)
    instances), computed in CHUNKED RETENTION form (chunk L=128):
      o[n] = sum_m g^|n-m| * s * (q[n].k[m]) * v[m]
      intra-chunk: (qf.kf) * D_local[128x128] then @ v
      cross-chunk: per-chunk state G[c][64dk,64dv], G = g^128*G + L(c),
      L(c) = sum_m' g^(127-m') k[m'] v[m']^T (fwd; mirrored bwd), applied as
      (q * s*g^(n'+1)) @ G[c].  This cuts attention FLOPs ~4x and the
      PSUM->SBUF attention-matrix traffic ~8x vs full N^2.
  - comm: AllToAll of k|v spikes per t (row->head shard) and of retention
    spikes per t (head->row shard). Spikes binary -> bf16 transport exact.
  - BatchNorm folded into weights/biases on host; projection biases enter
    PSUM via a rank-1 (ones x bias) matmul so LIF accumulate stays 1 DVE op.
  - LIF in scaled-threshold form: vt~ = 2^t v_t accumulated in bf16, spike
    when vt~ >= 2^t v_th, hard reset via (v<th)*v.

Row indexing: global r = t*(B*N) + b*N + n;  per-core rr = t*(B*NL) + b*NL + nl
with n = 128*core + nl.  Attention instance index inst = t*B + b.
DMAs are batched via multi-dim APs and spread across SP/ACT/POOL DGE queues.
"""
import os
import sys
import numpy as np
import ml_dtypes

for _p in ("/root/.axon_site/_ro/trn_rl_repo", "/opt/trn_rl_repo"):
    if os.path.isdir(_p) and _p not in sys.path:
        sys.path.append(_p)

bf16 = ml_dtypes.bfloat16

T, B, N, C = 4, 4, 1024, 512
H, D = 8, 64
HID = 4 * C
NCORES = 8
NL = N // NCORES          # 128
RR = T * B * NL           # 2048 rows per core
RF = T * B * N            # 16384 full rows
NI = T * B                # 16 attention instances
NC8 = 8                   # chunks per sequence (N / 128)
TAU, EPS = 2.0, 1e-5
SCALE = D ** -0.5

LAST_EXEC_NS = None
_CACHED = None


def _fold_bn(W, bias, g, beta, rm, rv):
    ghat = (np.asarray(g, np.float64) / np.sqrt(np.asarray(rv, np.float64) + EPS))
    Wf = (np.asarray(W, np.float64) * ghat[None, :]).astype(np.float32)
    bf_ = ((np.asarray(bias, np.float64) - np.asarray(rm, np.float64)) * ghat
           + np.asarray(beta, np.float64)).astype(np.float32)
    return Wf, bf_


def _build_nc():
    import concourse.bass as bass  # noqa: F401
    import concourse.bacc as bacc
    import concourse.mybir as mybir
    from concourse import tile

    f32 = mybir.dt.float32
    b16 = mybir.dt.bfloat16
    ALU = mybir.AluOpType
    ACT = mybir.ActivationFunctionType

    nc = bacc.Bacc("TRN2", target_bir_lowering=False, debug=False,
                   num_devices=NCORES)

    # ---- external inputs (per-core values via in_maps) ----
    xT3 = nc.declare_dram_parameter("xT3", [128, 4, RF], b16, isOutput=False)
    xrT = nc.declare_dram_parameter("xrT", [C, RR], b16, isOutput=False)
    xrf = nc.declare_dram_parameter("xrf", [C, RR], f32, isOutput=False)
    wqk = nc.declare_dram_parameter("wqk", [C, 128], b16, isOutput=False)
    bqk = nc.declare_dram_parameter("bqk", [128, T], f32, isOutput=False)
    wkv = nc.declare_dram_parameter("wkv", [C, 2 * C], b16, isOutput=False)
    bkv = nc.declare_dram_parameter("bkv", [1, 2 * C], b16, isOutput=False)
    wp = nc.declare_dram_parameter("wp", [C, C], b16, isOutput=False)
    bp = nc.declare_dram_parameter("bp", [128, 4 * T], f32, isOutput=False)
    w1 = nc.declare_dram_parameter("w1", [C, HID], b16, isOutput=False)
    b1 = nc.declare_dram_parameter("b1", [128, 16 * T], f32, isOutput=False)
    w2 = nc.declare_dram_parameter("w2", [HID, C], b16, isOutput=False)
    b2 = nc.declare_dram_parameter("b2", [128, 4 * T], f32, isOutput=False)
    dl = nc.declare_dram_parameter("dl", [128, 128], b16, isOutput=False)
    gq2 = nc.declare_dram_parameter("gq2", [64, 256], b16, isOutput=False)
    gv2 = nc.declare_dram_parameter("gv2", [128, 2], f32, isOutput=False)
    g128 = nc.declare_dram_parameter("g128", [128, 1], f32, isOutput=False)
    out_e = nc.declare_dram_parameter("out", [C, RR], f32, isOutput=True)

    # ---- internal DRAM ----
    RQ = RR // 4  # 512 rows per t
    u8 = mybir.dt.uint8
    kv_in = [nc.dram_tensor(f"kv_in{t}", [NCORES, RQ, 128], u8)
             for t in range(T)]
    kv_out = [nc.dram_tensor(f"kv_out{t}", [NCORES, RQ, 128], u8)
              for t in range(T)]
    rc_in = [nc.dram_tensor(f"rc_in{t}", [NCORES, D, RQ], u8) for t in range(T)]
    rc_out = [nc.dram_tensor(f"rc_out{t}", [NCORES, D, RQ], u8)
              for t in range(T)]
    ao_d = nc.dram_tensor("ao_d", [C, RR], b16)   # attn_out spikes bounce

    rg = [list(range(NCORES))]

    with tile.TileContext(nc) as tc:
      with tc.tile_pool(name="glob", bufs=1) as GP:
        SPH = {}  # phase-local spike/tmp pool, set below
        # ---------- global SBUF tensors (all weights preload at t=0) ----------
        xrT_sb = GP.tile([128, 4 * RR], b16, tag="xrT_sb")
        for kk in range(4):
            nc.gpsimd.dma_start(out=xrT_sb[:, kk * RR:(kk + 1) * RR],
                                in_=xrT[kk * 128:(kk + 1) * 128, :])
        wkv_sb = GP.tile([128, 4 * 2 * C], b16, tag="wkv_sb")
        for kk in range(4):
            nc.gpsimd.dma_start(out=wkv_sb[:, kk * 2 * C:(kk + 1) * 2 * C],
                                in_=wkv[kk * 128:(kk + 1) * 128, :])
        wqk_sb = GP.tile([128, 4 * 128], b16, tag="wqk_sb")
        for kk in range(4):
            nc.gpsimd.dma_start(out=wqk_sb[:, kk * 128:(kk + 1) * 128],
                                in_=wqk[kk * 128:(kk + 1) * 128, :])
        bqk_sb = GP.tile([128, T], f32, tag="bqk_sb")
        nc.gpsimd.dma_start(out=bqk_sb[:], in_=bqk[:])
        bkv_sb = GP.tile([1, 2 * C], b16, tag="bkv_sb")
        nc.gpsimd.dma_start(out=bkv_sb[:], in_=bkv[:])
        ones_sb = GP.tile([1, 128], b16, tag="ones_sb")
        nc.vector.memset(ones_sb[:], 1.0)
        dl_sb = GP.tile([128, 128], b16, tag="dl_sb")
        nc.gpsimd.dma_start(out=dl_sb[:], in_=dl[:])
        gq2_sb = GP.tile([64, 256], b16, tag="gq2_sb")
        nc.gpsimd.dma_start(out=gq2_sb[:], in_=gq2[:])
        gv2_sb = GP.tile([128, 2], f32, tag="gv2_sb")
        nc.gpsimd.dma_start(out=gv2_sb[:], in_=gv2[:])
        g128_sb = GP.tile([128, 1], f32, tag="g128_sb")
        nc.gpsimd.dma_start(out=g128_sb[:], in_=g128[:])
        wp_sb = GP.tile([128, 4 * C], b16, tag="wp_sb")
        for kk in range(4):
            nc.gpsimd.dma_start(out=wp_sb[:, kk * C:(kk + 1) * C],
                                in_=wp[kk * 128:(kk + 1) * 128, :])
        bp_sb = GP.tile([128, 4 * T], f32, tag="bp_sb")
        nc.gpsimd.dma_start(out=bp_sb[:], in_=bp[:])
        b1_sb = GP.tile([128, 16 * T], f32, tag="b1_sb")
        nc.gpsimd.dma_start(out=b1_sb[:], in_=b1[:])
        b2_sb = GP.tile([128, 4 * T], f32, tag="b2_sb")
        nc.gpsimd.dma_start(out=b2_sb[:], in_=b2[:])

        def lif_step(psum_ap, state_ap, spike_ap, t, vth, nparts,
                     bias_ap=None, last_t=T - 1):
            sc = float(2.0 ** (t - 1))
            th = float((2.0 ** t) * vth)
            fd = psum_ap.shape[-1]
            if t == 0:
                if bias_ap is not None:
                    nc.scalar.activation(state_ap, psum_ap, ACT.Identity,
                                         bias=bias_ap, scale=sc)
                else:
                    nc.scalar.activation(state_ap, psum_ap, ACT.Copy, scale=sc)
            else:
                pt = SPH["p"].tile([128, fd], b16, tag="lif_tmp")
                pa = pt[0:nparts, :]
                if bias_ap is not None:
                    nc.scalar.activation(pa, psum_ap, ACT.Identity,
                                         bias=bias_ap, scale=sc)
                else:
                    nc.scalar.activation(pa, psum_ap, ACT.Copy, scale=sc)
                nc.vector.tensor_add(state_ap, state_ap, pa)
            nc.gpsimd.tensor_single_scalar(spike_ap, state_ap, th, ALU.is_ge)
            if t < last_t:
                nc.vector.scalar_tensor_tensor(state_ap, state_ap, th,
                                               state_ap, ALU.is_lt, ALU.mult)

        def lif_pair(psums, biases, state_ap, spike_ap, t, vth,
                     last_t=T - 1):
            """Adjacent psums -> one widened DVE chain (ACT per psum)."""
            sc = float(2.0 ** (t - 1))
            th = float((2.0 ** t) * vth)
            fd = state_ap.shape[-1]
            if t == 0:
                off = 0
                for ps, bias in zip(psums, biases):
                    w = ps.shape[-1]
                    if bias is not None:
                        nc.scalar.activation(state_ap[:, off:off + w], ps,
                                             ACT.Identity, bias=bias, scale=sc)
                    else:
                        nc.scalar.activation(state_ap[:, off:off + w], ps,
                                             ACT.Copy, scale=sc)
                    off += w
            else:
                pt = SPH["p"].tile([128, fd], b16, tag="lif_tmp")
                off = 0
                for ps, bias in zip(psums, biases):
                    w = ps.shape[-1]
                    if bias is not None:
                        nc.scalar.activation(pt[:, off:off + w], ps,
                                             ACT.Identity, bias=bias, scale=sc)
                    else:
                        nc.scalar.activation(pt[:, off:off + w], ps,
                                             ACT.Copy, scale=sc)
                    off += w
                nc.vector.tensor_add(state_ap, state_ap, pt[:])
            nc.gpsimd.tensor_single_scalar(spike_ap, state_ap, th, ALU.is_ge)
            if t < last_t:
                nc.vector.scalar_tensor_tensor(state_ap, state_ap, th,
                                               state_ap, ALU.is_lt, ALU.mult)

        # ================= phase 1: projections + attention =================
        with (
            tc.tile_pool(name="ph1", bufs=1) as P1,
            tc.tile_pool(name="spike1", bufs=3) as SP1,
        ):
            SPH["p"] = SP1
            qk_sp = P1.tile([128, RF], b16, tag="qk_sp")   # [q_h; k_h] spikes
            kv_st = P1.tile([128, B * 2 * C], b16, tag="kv_st")
            rv_st = P1.tile([64, B * N], b16, tag="rv_st")

            # ----- stage B: k,v projection (row-shard, row-major) -----
            with tc.tile_pool(name="psb", bufs=2, space="PSUM") as PSB:
                for inst in range(NI):
                    t, bb = inst // B, inst % B
                    ps = PSB.tile([128, 2 * C], f32, tag="psb")
                    for half in range(2):
                        hof = half * C
                        nc.tensor.matmul(ps[:, hof:hof + C], ones_sb[:],
                                         bkv_sb[:, hof:hof + C],
                                         start=True, stop=False)
                        for kk in range(4):
                            nc.tensor.matmul(
                                ps[:, hof:hof + C],
                                xrT_sb[:, kk * RR + inst * 128:
                                       kk * RR + (inst + 1) * 128],
                                wkv_sb[:, kk * 2 * C + hof:
                                       kk * 2 * C + hof + C],
                                start=False, stop=(kk == 3))
                    sp_kv = SP1.tile([128, 2 * C], u8, tag="kv_spike")
                    lif_pair([ps[:, 0:C], ps[:, C:2 * C]], [None, None],
                             kv_st[:, bb * 2 * C:(bb + 1) * 2 * C], sp_kv[:],
                             t, 1.0)
                    nc.sync.dma_start(
                        out=kv_in[t][:, bb * 128:(bb + 1) * 128,
                                     :].transpose([1, 0, 2]),
                        in_=sp_kv[:].rearrange("p (h f) -> p h f", h=8))
                    if bb == B - 1:
                        nc.gpsimd.collective_compute(
                            "AllToAll", ALU.bypass, replica_groups=rg,
                            ins=[kv_in[t][:]], outs=[kv_out[t][:]])

            # ----- stage A: q,k projection (col-shard, feature-major) -----
            with (
                tc.tile_pool(name="psa2", bufs=2, space="PSUM") as PSA2,
                tc.tile_pool(name="xa", bufs=3) as XA,
                tc.tile_pool(name="qkst", bufs=1) as QKST,
            ):
                qk_st = QKST.tile([128, RF // T], b16, tag="qk_st")
                for fp in range(RF // 1024):        # fi pairs (same t)
                    t = fp // 4
                    pss = []
                    for j in range(2):
                        fi = 2 * fp + j
                        xt_ = XA.tile([128, 2048], b16, tag="xa")
                        nc.scalar.dma_start(
                            out=xt_[:],
                            in_=xT3[:, :, fi * 512:(fi + 1) * 512])
                        ps = PSA2.tile([128, 512], f32, tag="psa2")
                        for kk in range(4):
                            nc.tensor.matmul(
                                ps[:],
                                wqk_sb[:, kk * 128:(kk + 1) * 128],
                                xt_[:, kk * 512:(kk + 1) * 512],
                                start=(kk == 0), stop=(kk == 3))
                        pss.append(ps)
                    pos = ((2 * fp) % 8) * 512
                    bcol = bqk_sb[:, t:t + 1]
                    lif_pair([p[:] for p in pss], [bcol, bcol],
                             qk_st[:, pos:pos + 1024],
                             qk_sp[:, fp * 1024:(fp + 1) * 1024], t, 1.0)

            # ----- stage D: chunked retention attention (head-shard) -----
            with (
                tc.tile_pool(name="psl", bufs=2, space="PSUM") as PSL,
                tc.tile_pool(name="psat", bufs=2, space="PSUM") as PSAT,
                tc.tile_pool(name="pso2", bufs=3, space="PSUM") as PSO2,
                tc.tile_pool(name="dw", bufs=1) as DW,
                tc.tile_pool(name="dwork", bufs=2) as DK,
            ):
                for t in range(T):
                    # --- D1: chunk states G via L matmuls ---
                    ktg_u = DK.tile([128, 2048], u8, tag="ktg_u")
                    nc.sync.dma_start(
                        out=ktg_u[:].rearrange("p (q d) -> p q d", d=64),
                        in_=kv_out[t][:, :, 0:64].rearrange(
                            "c (b p) d -> p (c b) d", b=4))
                    vtg_u = DK.tile([128, 2048], u8, tag="vtg_u")
                    nc.sync.dma_start(
                        out=vtg_u[:].rearrange("p (q d) -> p q d", d=64),
                        in_=kv_out[t][:, :, 64:128].rearrange(
                            "c (b p) d -> p (c b) d", b=4))
                    ktg = DK.tile([128, 2048], b16, tag="ktg")
                    nc.gpsimd.tensor_copy(ktg[:], ktg_u[:])
                    vtg = DK.tile([128, 2048], b16, tag="vtg")
                    nc.gpsimd.tensor_copy(vtg[:], vtg_u[:])
                    gbf = {}
                    gbb = {}
                    for dirn in range(2):
                        grun = DK.tile([64, 256], f32, tag=f"grun{dirn}")
                        nc.vector.memset(grun[:], 0.0)
                        crange = (range(NC8) if dirn == 0
                                  else range(NC8 - 1, -1, -1))
                        for c in crange:
                            kt = DK.tile([128, 256], b16, tag="kt")
                            nc.scalar.activation(
                                kt[:], ktg[:, c * 256:(c + 1) * 256],
                                ACT.Copy, scale=gv2_sb[:, dirn:dirn + 1])
                            lp = PSL.tile([64, 256], f32, tag="lp")
                            for b in range(B):
                                vof = (c * 4 + b) * 64
                                nc.tensor.matmul(
                                    lp[:, b * 64:(b + 1) * 64],
                                    kt[:, b * 64:(b + 1) * 64],
                                    vtg[:, vof:vof + 64],
                                    start=True, stop=True)
                            snap = (c > 0) if dirn == 0 else (c < NC8 - 1)
                            if snap:
                                gb = DK.tile([64, 256], b16,
                                             tag=f"gb{dirn}_{c}")
                                nc.scalar.copy(gb[:], grun[:])
                                (gbf if dirn == 0 else gbb)[c] = gb
                            nc.vector.scalar_tensor_tensor(
                                grun[:], grun[:], g128_sb[0:64, 0:1], lp[:],
                                ALU.mult, ALU.add)
                    # --- D3: intra + cross, retention LIF ---
                    ks = DK.tile([64, 4096], b16, tag="ks", bufs=1)
                    nc.sync.dma_start(
                        out=ks[:],
                        in_=qk_sp[64:128, t * 4096:(t + 1) * 4096])
                    qts = []
                    for dirn in range(2):
                        qt = DK.tile([64, 4096], b16, tag=f"qt{dirn}", bufs=1)
                        nc.vector.tensor_mul(
                            qt[:].rearrange("p (x n) -> p x n", n=128),
                            qk_sp[0:64, t * 4096:(t + 1) * 4096].rearrange(
                                "p (x n) -> p x n", n=128),
                            gq2_sb[:, dirn * 128:(dirn + 1) * 128].unsqueeze(
                                1).broadcast_to([64, 32, 128]))
                        qts.append(qt)
                    for cp in range(4):
                        ops = []
                        for j in range(2):
                            c = 2 * cp + j
                            atp = PSAT.tile([128, 512], f32, tag="atp")
                            for b in range(B):
                                qof = (4 * t + b) * 1024 + c * 128
                                nc.tensor.matmul(
                                    atp[:, b * 128:(b + 1) * 128],
                                    ks[:, b * 1024 + c * 128:
                                       b * 1024 + (c + 1) * 128],
                                    qk_sp[0:64, qof:qof + 128],
                                    start=True, stop=True)
                            ats = DK.tile([128, 512], b16, tag="ats")
                            nc.vector.tensor_mul(
                                ats[:].rearrange("p (b n) -> p b n", b=4),
                                atp[:].rearrange("p (b n) -> p b n", b=4),
                                dl_sb[:].unsqueeze(1).broadcast_to(
                                    [128, 4, 128]))
                            op = PSO2.tile([64, 512], f32, tag="op")
                            nmm = []
                            for b in range(B):
                                nmm.append(("av", b))
                                if c > 0:
                                    nmm.append(("f", b))
                                if c < NC8 - 1:
                                    nmm.append(("bw", b))
                            for i, (kind, b) in enumerate(nmm):
                                st_ = (i == 0)
                                sp_ = (i == len(nmm) - 1)
                                oseg = op[:, b * 128:(b + 1) * 128]
                                if kind == "av":
                                    vof = (c * 4 + b) * 64
                                    nc.tensor.matmul(
                                        oseg, vtg[:, vof:vof + 64],
                                        ats[:, b * 128:(b + 1) * 128],
                                        start=st_, stop=sp_)
                                elif kind == "f":
                                    nc.tensor.matmul(
                                        oseg,
                                        gbf[c][:, b * 64:(b + 1) * 64],
                                        qts[0][:, b * 1024 + c * 128:
                                               b * 1024 + (c + 1) * 128],
                                        start=st_, stop=sp_)
                                else:
                                    nc.tensor.matmul(
                                        oseg,
                                        gbb[c][:, b * 64:(b + 1) * 64],
                                        qts[1][:, b * 1024 + c * 128:
                                               b * 1024 + (c + 1) * 128],
                                        start=st_, stop=sp_)
                            ops.append(op)
                        # retention LIF over the chunk pair
                        sc = float(2.0 ** (t - 1))
                        th = float((2.0 ** t) * 0.5)
                        stv = rv_st[:].rearrange("p (b n) -> p b n", b=4)[
                            :, :, cp * 256:(cp + 1) * 256]
                        if t == 0:
                            # ACT writes straight into the (strided) state view
                            for j in range(2):
                                c = 2 * cp + j
                                stc = rv_st[:].rearrange(
                                    "p (b n) -> p b n", b=4)[
                                    :, :, c * 128:(c + 1) * 128]
                                nc.scalar.activation(
                                    stc,
                                    ops[j][:].rearrange(
                                        "p (b n) -> p b n", b=4),
                                    ACT.Copy, scale=sc)
                        else:
                            pa = SP1.tile([64, 1024], b16, tag="ret_tmp")
                            pav = pa[:].rearrange("p (b c n) -> p b c n",
                                                  b=4, c=2)
                            for j in range(2):
                                nc.scalar.activation(
                                    pav[:, :, j:j + 1, :].squeeze(2),
                                    ops[j][:].rearrange(
                                        "p (b n) -> p b n", b=4),
                                    ACT.Copy, scale=sc)
                            nc.vector.tensor_add(
                                stv, stv, pa[:].rearrange(
                                    "p (b n) -> p b n", b=4))
                        rs = SP1.tile([64, 1024], u8, tag="r_spike")
                        rsv = rs[:].rearrange("p (b n) -> p b n", b=4)
                        nc.gpsimd.tensor_single_scalar(rsv, stv, th, ALU.is_ge)
                        if t < T - 1:
                            nc.vector.scalar_tensor_tensor(
                                stv, stv, th, stv, ALU.is_lt, ALU.mult)
                        for j in range(2):
                            nc.sync.dma_start(
                                out=rc_in[t][2 * cp + j, :, :].rearrange(
                                    "p (b n) -> p b n", b=4),
                                in_=rs[:].rearrange(
                                    "p (b c n) -> p b c n", b=4,
                                    c=2)[:, :, j:j + 1, :].squeeze(2))
                    nc.gpsimd.collective_compute(
                        "AllToAll", ALU.bypass, replica_groups=rg,
                        ins=[rc_in[t][:]], outs=[rc_out[t][:]])

        # ================= phase 2: out-proj + MLP (row-shard) =================
        with (
            tc.tile_pool(name="ph2", bufs=1) as P2,
            tc.tile_pool(name="spike2", bufs=3) as SP2,
            tc.tile_pool(name="work", bufs=3) as WP,
        ):
            SPH["p"] = SP2
            w2_sb = P2.tile([128, 16 * C], b16, tag="w2_sb")
            for kk in range(16):
                nc.gpsimd.dma_start(out=w2_sb[:, kk * C:(kk + 1) * C],
                                    in_=w2[kk * 128:(kk + 1) * 128, :])
            w1_sb = P2.tile([128, 4 * HID], b16, tag="w1_sb")
            for kk in range(4):
                nc.gpsimd.dma_start(out=w1_sb[:, kk * HID:(kk + 1) * HID],
                                    in_=w1[kk * 128:(kk + 1) * 128, :])
            p_st = P2.tile([128, 4 * 512], b16, tag="p_st")
            h_st = P2.tile([128, 16 * 512], b16, tag="h_st")
            m_st = P2.tile([128, 4 * 512], b16, tag="m_st")
            x1 = P2.tile([128, 4 * RR], b16, tag="x1")

            with tc.tile_pool(name="psh", bufs=2, space="PSUM") as PSH:
                # ----- stage F: output projection + residual 1 -----
                for fc in range(4):          # fc == t (512-row slabs)
                    rts = []
                    for kk in range(4):
                        rtu = WP.tile([128, 512], mybir.dt.uint8,
                                      tag="wp_rhs_u", bufs=4)
                        nc.scalar.dma_start(out=rtu[0:64, :],
                                            in_=rc_out[fc][2 * kk, :, :])
                        nc.scalar.dma_start(out=rtu[64:128, :],
                                            in_=rc_out[fc][2 * kk + 1, :, :])
                        rt = WP.tile([128, 512], b16, tag="wp_rhs", bufs=8)
                        nc.gpsimd.tensor_copy(rt[:], rtu[:])
                        rts.append(rt)
                    for cp in range(2):      # cc pairs
                        ph2t = PSH.tile([128, 1024], f32, tag="psh")
                        for j in range(2):
                            cc = 2 * cp + j
                            for kk in range(4):
                                nc.tensor.matmul(
                                    ph2t[:, j * 512:(j + 1) * 512],
                                    wp_sb[:, kk * C + cc * 128:
                                          kk * C + (cc + 1) * 128],
                                    rts[kk][:], start=(kk == 0), stop=(kk == 3))
                        sp2 = SP2.tile([128, 1024], b16, tag="p_spike")
                        cc0 = 2 * cp
                        lif_pair(
                            [ph2t[:, 0:512], ph2t[:, 512:1024]],
                            [bp_sb[:, cc0 * T + fc:cc0 * T + fc + 1],
                             bp_sb[:, (cc0 + 1) * T + fc:(cc0 + 1) * T + fc + 1]],
                            p_st[:, cc0 * 512:(cc0 + 2) * 512], sp2[:], fc, 1.0)
                        nc.sync.dma_start(
                            out=ao_d[cc0 * 128:(cc0 + 2) * 128,
                                     fc * 512:(fc + 1) * 512].rearrange(
                                "(j p) c -> p j c", j=2),
                            in_=sp2[:].rearrange("p (j c) -> p j c", j=2))
                        nc.vector.tensor_add(
                            x1[:].rearrange("p (cc r) -> p cc r", cc=4)[
                                :, cc0:cc0 + 2, fc * 512:(fc + 1) * 512],
                            sp2[:].rearrange("p (j c) -> p j c", j=2),
                            xrT_sb[:].rearrange("p (cc r) -> p cc r", cc=4)[
                                :, cc0:cc0 + 2, fc * 512:(fc + 1) * 512])

                # ----- stage H: MLP + residual 2 -----
                with tc.tile_pool(name="psm", bufs=1, space="PSUM") as PSM:
                    for rq in range(4):      # rq == t
                        pm = [PSM.tile([128, 1024], f32, tag=f"pm{cp}",
                                       name=f"pm{cp}_{rq}") for cp in range(2)]
                        for hp in range(8):          # hc pairs
                            ph = PSH.tile([128, 1024], f32, tag="psh")
                            for j in range(2):
                                hc = 2 * hp + j
                                for kk in range(4):
                                    nc.tensor.matmul(
                                        ph[:, j * 512:(j + 1) * 512],
                                        w1_sb[:, kk * HID + hc * 128:
                                              kk * HID + (hc + 1) * 128],
                                        x1[:, kk * RR + rq * 512:
                                           kk * RR + (rq + 1) * 512],
                                        start=(kk == 0), stop=(kk == 3))
                            hs = SP2.tile([128, 1024], b16, tag="h_spike")
                            hc0 = 2 * hp
                            lif_pair(
                                [ph[:, 0:512], ph[:, 512:1024]],
                                [b1_sb[:, hc0 * T + rq:hc0 * T + rq + 1],
                                 b1_sb[:, (hc0 + 1) * T + rq:
                                       (hc0 + 1) * T + rq + 1]],
                                h_st[:, hc0 * 512:(hc0 + 2) * 512], hs[:],
                                rq, 1.0)
                            for j in range(2):
                                hc = 2 * hp + j
                                for cc in range(4):
                                    nc.tensor.matmul(
                                        pm[cc // 2][:, (cc % 2) * 512:
                                                    (cc % 2) * 512 + 512],
                                        w2_sb[:, hc * C + cc * 128:
                                              hc * C + (cc + 1) * 128],
                                        hs[:, j * 512:(j + 1) * 512],
                                        start=(hc == 0), stop=(hc == 15))
                        for cp in range(2):
                            cc0 = 2 * cp
                            ms2 = SP2.tile([128, 1024], b16, tag="m_spike")
                            lif_pair(
                                [pm[cp][:, 0:512], pm[cp][:, 512:1024]],
                                [b2_sb[:, cc0 * T + rq:cc0 * T + rq + 1],
                                 b2_sb[:, (cc0 + 1) * T + rq:
                                       (cc0 + 1) * T + rq + 1]],
                                m_st[:, cc0 * 512:(cc0 + 2) * 512], ms2[:],
                                rq, 1.0)
                            xf = WP.tile([128, 1024], f32, tag="xf")
                            nc.sync.dma_start(
                                out=xf[:].rearrange("p (j c) -> p j c", j=2),
                                in_=xrf[cc0 * 128:(cc0 + 2) * 128,
                                        rq * 512:(rq + 1) * 512].rearrange(
                                    "(j p) c -> p j c", j=2))
                            av = WP.tile([128, 1024], b16, tag="av")
                            nc.sync.dma_start(
                                out=av[:].rearrange("p (j c) -> p j c", j=2),
                                in_=ao_d[cc0 * 128:(cc0 + 2) * 128,
                                         rq * 512:(rq + 1) * 512].rearrange(
                                    "(j p) c -> p j c", j=2))
                            ot = WP.tile([128, 1024], f32, tag="ot")
                            nc.vector.tensor_add(ot[:], xf[:], av[:])
                            nc.vector.tensor_add(ot[:], ot[:], ms2[:])
                            nc.sync.dma_start(
                                out=out_e[cc0 * 128:(cc0 + 2) * 128,
                                          rq * 512:(rq + 1) * 512].rearrange(
                                    "(j p) c -> p j c", j=2),
                                in_=ot[:].rearrange("p (j c) -> p j c", j=2))

    nc.compile()
    return nc


def _host_prep(inputs):
    x = np.asarray(inputs["x"], np.float32)          # (T,B,N,C)
    xT_b = x.transpose(3, 0, 1, 2).reshape(C, RF).astype(bf16)
    xT3_b = xT_b.reshape(4, 128, RF).transpose(1, 0, 2).copy()

    Wq, bq_ = _fold_bn(inputs["Wq"], inputs["bq"], inputs["gq"],
                       inputs["betaq"], inputs["rmq"], inputs["rvq"])
    Wk, bk_ = _fold_bn(inputs["Wk"], inputs["bk"], inputs["gk"],
                       inputs["betak"], inputs["rmk"], inputs["rvk"])
    Wv, bv_ = _fold_bn(inputs["Wv"], inputs["bv"], inputs["gv"],
                       inputs["betav"], inputs["rmv"], inputs["rvv"])
    Wp, bp_ = _fold_bn(inputs["Wp"], inputs["bp"], inputs["gp"],
                       inputs["betap"], inputs["rmp"], inputs["rvp"])
    W1, b1_ = _fold_bn(inputs["W1"], inputs["b1"], inputs["g1"],
                       inputs["beta1"], inputs["rm1"], inputs["rv1"])
    W2, b2_ = _fold_bn(inputs["W2"], inputs["b2"], inputs["g2"],
                       inputs["beta2"], inputs["rm2"], inputs["rv2"])

    tsc = np.array([2.0 ** (t - 1) for t in range(T)], np.float32)

    def pack_bias(bvec, nchunk):
        out = np.zeros((128, nchunk * T), np.float32)
        for ch in range(nchunk):
            for t in range(T):
                out[:, ch * T + t] = tsc[t] * bvec[ch * 128:(ch + 1) * 128]
        return out

    # k|v interleave: feature block hh*128 + (0:64 -> k head hh, 64:128 -> v)
    wkv_m = np.zeros((C, 2 * C), np.float32)
    bkv_m = np.zeros((1, 2 * C), np.float32)
    for hh in range(H):
        wkv_m[:, hh * 128:hh * 128 + 64] = Wk[:, hh * 64:(hh + 1) * 64]
        wkv_m[:, hh * 128 + 64:hh * 128 + 128] = Wv[:, hh * 64:(hh + 1) * 64]
        bkv_m[0, hh * 128:hh * 128 + 64] = bk_[hh * 64:(hh + 1) * 64]
        bkv_m[0, hh * 128 + 64:hh * 128 + 128] = bv_[hh * 64:(hh + 1) * 64]

    gamma = (1.0 - 2.0 ** (-5.0 - np.arange(H, dtype=np.float64)))
    idx = np.arange(128, dtype=np.float64)

    in_maps = []
    for cid in range(NCORES):
        h = cid
        g = gamma[h]
        dl_m = (SCALE * g ** np.abs(idx[:, None] - idx[None, :])).astype(
            np.float32)
        gq2_m = np.zeros((64, 256), np.float32)
        gq2_m[:, 0:128] = SCALE * g ** (idx + 1.0)[None, :]
        gq2_m[:, 128:256] = SCALE * g ** (127.0 - idx)[None, :]
        gv2_m = np.zeros((128, 2), np.float32)
        gv2_m[:, 0] = g ** (127.0 - idx)
        gv2_m[:, 1] = g ** (idx + 1.0)
        g128_m = np.full((128, 1), g ** 128.0, np.float32)

        xs = x[:, :, 128 * cid:128 * (cid + 1), :]       # (T,B,128,C)
        xrT_f = xs.transpose(3, 0, 1, 2).reshape(C, RR)

        wqk_m = np.concatenate([Wq[:, h * D:(h + 1) * D],
                                Wk[:, h * D:(h + 1) * D]], axis=1)
        bqk_m = np.zeros((128, T), np.float32)
        for t in range(T):
            bqk_m[0:64, t] = tsc[t] * bq_[h * D:(h + 1) * D]
            bqk_m[64:128, t] = tsc[t] * bk_[h * D:(h + 1) * D]

        in_maps.append({
            "xT3": xT3_b,
            "xrT": xrT_f.astype(bf16),
            "xrf": xrT_f.astype(np.float32),
            "wqk": wqk_m.astype(bf16),
            "bqk": bqk_m,
            "wkv": wkv_m.astype(bf16),
            "bkv": bkv_m.astype(bf16),
            "wp": Wp.astype(bf16),
            "bp": pack_bias(bp_, 4),
            "w1": W1.astype(bf16),
            "b1": pack_bias(b1_, 16),
            "w2": W2.astype(bf16),
            "b2": pack_bias(b2_, 4),
            "dl": dl_m.astype(bf16),
            "gq2": gq2_m.astype(bf16),
            "gv2": gv2_m,
            "g128": g128_m,
        })
    return in_maps


def _install_trace_hook():
    import types
    import antenv
    if "antenv.axon_hooks" in sys.modules:
        return True
    mod = types.ModuleType("antenv.axon_hooks")
    _h = [None]
    mod.set_axon_ntff_profile_hook = lambda hk: _h.__setitem__(0, hk)
    mod.get_axon_ntff_profile_hook = lambda: _h[0]
    sys.modules["antenv.axon_hooks"] = mod
    antenv.axon_hooks = mod
    try:
        from trn_agent_boot.trn_boot import _ntff_profile_via_ctypes
        hook = _ntff_profile_via_ctypes("/opt/axon/libaxon_pjrt.so")
        mod.set_axon_ntff_profile_hook(hook)
        return hook is not None
    except Exception:
        return False


def kernel(**inputs):
    global LAST_EXEC_NS, _CACHED
    from concourse.bass_utils import run_bass_kernel_spmd

    trace = os.environ.get("BASS_KERNEL_TRACE", "0") == "1"
    if trace:
        _install_trace_hook()

    if _CACHED is None:
        _CACHED = _build_nc()
    nc = _CACHED

    in_maps = _host_prep(inputs)
    res = run_bass_kernel_spmd(nc, in_maps, core_ids=list(range(NCORES)),
                               trace=trace)
    LAST_EXEC_NS = res.exec_time_ns

    full = np.empty((T, B, N, C), np.float32)
    for cid in range(NCORES):
        oc = res.results[cid]["out"]                    # (C, RR) f32
        full[:, :, 128 * cid:128 * (cid + 1), :] = (
            oc.reshape(C, T, B, NL).transpose(1, 2, 3, 0))
    return full


# revision 22
# speedup vs baseline: 3.7533x; 3.7533x over previous
"""Trainium2 Bass kernel for nn_Block_12154757448460 (spiking retention transformer).

Sharding over 8 NeuronCores (1 chip):
  - q,k projections: column-sharded by head (core h computes head h's 64 q-cols +
    64 k-cols for ALL rows) -> attention intra-chunk inputs stay local.
  - k,v projections (row-major copies), output projection, MLP: row-sharded
    (core c owns n in [128c, 128(c+1)) for all t,b).
  - attention + retention LIF: head-sharded (core h = head h, 16 (t,b)
    instances), computed in CHUNKED RETENTION form (chunk L=128):
      o[n] = sum_m g^|n-m| * s * (q[n].k[m]) * v[m]
      intra-chunk: (qf.kf) * D_local[128x128] then @ v
      cross-chunk: per-chunk state G[c][64dk,64dv], G = g^128*G + L(c),
      L(c) = sum_m' g^(127-m') k[m'] v[m']^T (fwd; mirrored bwd), applied as
      (q * s*g^(n'+1)) @ G[c].  This cuts attention FLOPs ~4x and the
      PSUM->SBUF attention-matrix traffic ~8x vs full N^2.
  - comm: AllToAll of k|v spikes per t (row->head shard) and of retention
    spikes per t (head->row shard). Spikes binary -> bf16 transport exact.
  - BatchNorm folded into weights/biases on host; projection biases enter
    PSUM via a rank-1 (ones x bias) matmul so LIF accumulate stays 1 DVE op.
  - LIF in scaled-threshold form: vt~ = 2^t v_t accumulated in bf16, spike
    when vt~ >= 2^t v_th, hard reset via (v<th)*v.

Row indexing: global r = t*(B*N) + b*N + n;  per-core rr = t*(B*NL) + b*NL + nl
with n = 128*core + nl.  Attention instance index inst = t*B + b.
DMAs are batched via multi-dim APs and spread across SP/ACT/POOL DGE queues.
"""
import os
import sys
import numpy as np
import ml_dtypes

for _p in ("/root/.axon_site/_ro/trn_rl_repo", "/opt/trn_rl_repo"):
    if os.path.isdir(_p) and _p not in sys.path:
        sys.path.append(_p)

bf16 = ml_dtypes.bfloat16

T, B, N, C = 4, 4, 1024, 512
H, D = 8, 64
HID = 4 * C
NCORES = 8
NL = N // NCORES          # 128
RR = T * B * NL           # 2048 rows per core
RF = T * B * N            # 16384 full rows
NI = T * B                # 16 attention instances
NC8 = 8                   # chunks per sequence (N / 128)
TAU, EPS = 2.0, 1e-5
SCALE = D ** -0.5

LAST_EXEC_NS = None
_CACHED = None


def _fold_bn(W, bias, g, beta, rm, rv):
    ghat = (np.asarray(g, np.float64) / np.sqrt(np.asarray(rv, np.float64) + EPS))
    Wf = (np.asarray(W, np.float64) * ghat[None, :]).astype(np.float32)
    bf_ = ((np.asarray(bias, np.float64) - np.asarray(rm, np.float64)) * ghat
           + np.asarray(beta, np.float64)).astype(np.float32)
    return Wf, bf_


def _build_nc():
    import concourse.bass as bass  # noqa: F401
    import concourse.bacc as bacc
    import concourse.mybir as mybir
    from concourse import tile

    f32 = mybir.dt.float32
    b16 = mybir.dt.bfloat16
    ALU = mybir.AluOpType
    ACT = mybir.ActivationFunctionType

    nc = bacc.Bacc("TRN2", target_bir_lowering=False, debug=False,
                   num_devices=NCORES)

    # ---- external inputs (per-core values via in_maps) ----
    xT3 = nc.declare_dram_parameter("xT3", [128, 4, RF], b16, isOutput=False)
    xrT = nc.declare_dram_parameter("xrT", [C, RR], b16, isOutput=False)
    xrf = nc.declare_dram_parameter("xrf", [C, RR], f32, isOutput=False)
    wqk = nc.declare_dram_parameter("wqk", [C, 128], b16, isOutput=False)
    bqk = nc.declare_dram_parameter("bqk", [128, T], f32, isOutput=False)
    wkv = nc.declare_dram_parameter("wkv", [C, 2 * C], b16, isOutput=False)
    bkv = nc.declare_dram_parameter("bkv", [1, 2 * C], b16, isOutput=False)
    wp = nc.declare_dram_parameter("wp", [C, C], b16, isOutput=False)
    bp = nc.declare_dram_parameter("bp", [128, 4 * T], f32, isOutput=False)
    w1 = nc.declare_dram_parameter("w1", [C, HID], b16, isOutput=False)
    b1 = nc.declare_dram_parameter("b1", [128, 16 * T], f32, isOutput=False)
    w2 = nc.declare_dram_parameter("w2", [HID, C], b16, isOutput=False)
    b2 = nc.declare_dram_parameter("b2", [128, 4 * T], f32, isOutput=False)
    dl = nc.declare_dram_parameter("dl", [128, 128], b16, isOutput=False)
    gq2 = nc.declare_dram_parameter("gq2", [128, 128], b16, isOutput=False)
    gv2 = nc.declare_dram_parameter("gv2", [128, 2], f32, isOutput=False)
    g128 = nc.declare_dram_parameter("g128", [128, 1], f32, isOutput=False)
    out_e = nc.declare_dram_parameter("out", [C, RR], f32, isOutput=True)

    # ---- internal DRAM ----
    RQ = RR // 4  # 512 rows per t
    u8 = mybir.dt.uint8
    kv_in = [nc.dram_tensor(f"kv_in{t}", [NCORES, RQ, 128], u8)
             for t in range(T)]
    kv_out = [nc.dram_tensor(f"kv_out{t}", [NCORES, RQ, 128], u8)
              for t in range(T)]
    rc_in = [nc.dram_tensor(f"rc_in{t}", [NCORES, D, RQ], u8) for t in range(T)]
    rc_out = [nc.dram_tensor(f"rc_out{t}", [NCORES, D, RQ], u8)
              for t in range(T)]
    ao_d = nc.dram_tensor("ao_d", [C, RR], b16)   # attn_out spikes bounce

    rg = [list(range(NCORES))]

    with tile.TileContext(nc) as tc:
      with tc.tile_pool(name="glob", bufs=1) as GP:
        SPH = {}  # phase-local spike/tmp pool, set below
        # ---------- global SBUF tensors (all weights preload at t=0) ----------
        xrT_sb = GP.tile([128, 4 * RR], b16, tag="xrT_sb")
        for kk in range(4):
            nc.gpsimd.dma_start(out=xrT_sb[:, kk * RR:(kk + 1) * RR],
                                in_=xrT[kk * 128:(kk + 1) * 128, :])
        wkv_sb = GP.tile([128, 4 * 2 * C], b16, tag="wkv_sb")
        for kk in range(4):
            nc.gpsimd.dma_start(out=wkv_sb[:, kk * 2 * C:(kk + 1) * 2 * C],
                                in_=wkv[kk * 128:(kk + 1) * 128, :])
        wqk_sb = GP.tile([128, 4 * 128], b16, tag="wqk_sb")
        for kk in range(4):
            nc.gpsimd.dma_start(out=wqk_sb[:, kk * 128:(kk + 1) * 128],
                                in_=wqk[kk * 128:(kk + 1) * 128, :])
        bqk_sb = GP.tile([128, T], f32, tag="bqk_sb")
        nc.gpsimd.dma_start(out=bqk_sb[:], in_=bqk[:])
        bkv_sb = GP.tile([1, 2 * C], b16, tag="bkv_sb")
        nc.gpsimd.dma_start(out=bkv_sb[:], in_=bkv[:])
        ones_sb = GP.tile([1, 128], b16, tag="ones_sb")
        nc.vector.memset(ones_sb[:], 1.0)
        dl_sb = GP.tile([128, 128], b16, tag="dl_sb")
        nc.gpsimd.dma_start(out=dl_sb[:], in_=dl[:])
        gq2_sb = GP.tile([128, 128], b16, tag="gq2_sb")
        nc.gpsimd.dma_start(out=gq2_sb[:], in_=gq2[:])
        gv2_sb = GP.tile([128, 2], f32, tag="gv2_sb")
        nc.gpsimd.dma_start(out=gv2_sb[:], in_=gv2[:])
        g128_sb = GP.tile([128, 1], f32, tag="g128_sb")
        nc.gpsimd.dma_start(out=g128_sb[:], in_=g128[:])
        wp_sb = GP.tile([128, 4 * C], b16, tag="wp_sb")
        for kk in range(4):
            nc.gpsimd.dma_start(out=wp_sb[:, kk * C:(kk + 1) * C],
                                in_=wp[kk * 128:(kk + 1) * 128, :])
        bp_sb = GP.tile([128, 4 * T], f32, tag="bp_sb")
        nc.gpsimd.dma_start(out=bp_sb[:], in_=bp[:])
        b1_sb = GP.tile([128, 16 * T], f32, tag="b1_sb")
        nc.gpsimd.dma_start(out=b1_sb[:], in_=b1[:])
        b2_sb = GP.tile([128, 4 * T], f32, tag="b2_sb")
        nc.gpsimd.dma_start(out=b2_sb[:], in_=b2[:])

        def lif_step(psum_ap, state_ap, spike_ap, t, vth, nparts,
                     bias_ap=None, last_t=T - 1):
            sc = float(2.0 ** (t - 1))
            th = float((2.0 ** t) * vth)
            fd = psum_ap.shape[-1]
            if t == 0:
                if bias_ap is not None:
                    nc.scalar.activation(state_ap, psum_ap, ACT.Identity,
                                         bias=bias_ap, scale=sc)
                else:
                    nc.scalar.activation(state_ap, psum_ap, ACT.Copy, scale=sc)
            else:
                pt = SPH["p"].tile([128, fd], b16, tag="lif_tmp")
                pa = pt[0:nparts, :]
                if bias_ap is not None:
                    nc.scalar.activation(pa, psum_ap, ACT.Identity,
                                         bias=bias_ap, scale=sc)
                else:
                    nc.scalar.activation(pa, psum_ap, ACT.Copy, scale=sc)
                nc.vector.tensor_add(state_ap, state_ap, pa)
            nc.vector.tensor_single_scalar(spike_ap, state_ap, th, ALU.is_ge)
            if t < last_t:
                nc.vector.scalar_tensor_tensor(state_ap, state_ap, th,
                                               state_ap, ALU.is_lt, ALU.mult)

        def lif_pair(psums, biases, state_ap, spike_ap, t, vth,
                     last_t=T - 1):
            """Adjacent psums -> one widened DVE chain (ACT per psum)."""
            sc = float(2.0 ** (t - 1))
            th = float((2.0 ** t) * vth)
            fd = state_ap.shape[-1]
            if t == 0:
                off = 0
                for ps, bias in zip(psums, biases):
                    w = ps.shape[-1]
                    if bias is not None:
                        nc.scalar.activation(state_ap[:, off:off + w], ps,
                                             ACT.Identity, bias=bias, scale=sc)
                    else:
                        nc.scalar.activation(state_ap[:, off:off + w], ps,
                                             ACT.Copy, scale=sc)
                    off += w
            else:
                pt = SPH["p"].tile([128, fd], b16, tag="lif_tmp")
                off = 0
                for ps, bias in zip(psums, biases):
                    w = ps.shape[-1]
                    if bias is not None:
                        nc.scalar.activation(pt[:, off:off + w], ps,
                                             ACT.Identity, bias=bias, scale=sc)
                    else:
                        nc.scalar.activation(pt[:, off:off + w], ps,
                                             ACT.Copy, scale=sc)
                    off += w
                nc.vector.tensor_add(state_ap, state_ap, pt[:])
            nc.vector.tensor_single_scalar(spike_ap, state_ap, th, ALU.is_ge)
            if t < last_t:
                nc.vector.scalar_tensor_tensor(state_ap, state_ap, th,
                                               state_ap, ALU.is_lt, ALU.mult)

        # ================= phase 1: projections + attention =================
        with (
            tc.tile_pool(name="ph1", bufs=1) as P1,
            tc.tile_pool(name="spike1", bufs=3) as SP1,
        ):
            SPH["p"] = SP1
            qk_sp = P1.tile([128, RF], b16, tag="qk_sp")   # [q_h; k_h] spikes
            kv_st = P1.tile([128, B * 2 * C], b16, tag="kv_st")
            rv_st = P1.tile([64, B * N], b16, tag="rv_st")

            # ----- stage B: k,v projection (row-shard, row-major) -----
            with tc.tile_pool(name="psb", bufs=2, space="PSUM") as PSB:
                for inst in range(NI):
                    t, bb = inst // B, inst % B
                    ps = PSB.tile([128, 2 * C], f32, tag="psb")
                    for half in range(2):
                        hof = half * C
                        nc.tensor.matmul(ps[:, hof:hof + C], ones_sb[:],
                                         bkv_sb[:, hof:hof + C],
                                         start=True, stop=False)
                        for kk in range(4):
                            nc.tensor.matmul(
                                ps[:, hof:hof + C],
                                xrT_sb[:, kk * RR + inst * 128:
                                       kk * RR + (inst + 1) * 128],
                                wkv_sb[:, kk * 2 * C + hof:
                                       kk * 2 * C + hof + C],
                                start=False, stop=(kk == 3))
                    sp_kv = SP1.tile([128, 2 * C], u8, tag="kv_spike")
                    lif_pair([ps[:, 0:C], ps[:, C:2 * C]], [None, None],
                             kv_st[:, bb * 2 * C:(bb + 1) * 2 * C], sp_kv[:],
                             t, 1.0)
                    nc.sync.dma_start(
                        out=kv_in[t][:, bb * 128:(bb + 1) * 128,
                                     :].transpose([1, 0, 2]),
                        in_=sp_kv[:].rearrange("p (h f) -> p h f", h=8))
                    if bb == B - 1:
                        nc.gpsimd.collective_compute(
                            "AllToAll", ALU.bypass, replica_groups=rg,
                            ins=[kv_in[t][:]], outs=[kv_out[t][:]])

            # ----- stage A: q,k projection (col-shard, feature-major) -----
            with (
                tc.tile_pool(name="psa2", bufs=2, space="PSUM") as PSA2,
                tc.tile_pool(name="xa", bufs=3) as XA,
                tc.tile_pool(name="qkst", bufs=1) as QKST,
            ):
                qk_st = QKST.tile([128, RF // T], b16, tag="qk_st")
                for fp in range(RF // 1024):        # fi pairs (same t)
                    t = fp // 4
                    pss = []
                    for j in range(2):
                        fi = 2 * fp + j
                        xt_ = XA.tile([128, 2048], b16, tag="xa")
                        nc.scalar.dma_start(
                            out=xt_[:],
                            in_=xT3[:, :, fi * 512:(fi + 1) * 512])
                        ps = PSA2.tile([128, 512], f32, tag="psa2")
                        for kk in range(4):
                            nc.tensor.matmul(
                                ps[:],
                                wqk_sb[:, kk * 128:(kk + 1) * 128],
                                xt_[:, kk * 512:(kk + 1) * 512],
                                start=(kk == 0), stop=(kk == 3))
                        pss.append(ps)
                    pos = ((2 * fp) % 8) * 512
                    bcol = bqk_sb[:, t:t + 1]
                    lif_pair([p[:] for p in pss], [bcol, bcol],
                             qk_st[:, pos:pos + 1024],
                             qk_sp[:, fp * 1024:(fp + 1) * 1024], t, 1.0)

            # ----- stage D: chunked retention attention (head-shard) -----
            with (
                tc.tile_pool(name="psl", bufs=2, space="PSUM") as PSL,
                tc.tile_pool(name="psat", bufs=3, space="PSUM") as PSAT,
                tc.tile_pool(name="pso2", bufs=3, space="PSUM") as PSO2,
                tc.tile_pool(name="dw", bufs=1) as DW,
                tc.tile_pool(name="dwork", bufs=2) as DK,
            ):
                for t in range(T):
                    # --- D1: chunk states G via L matmuls ---
                    ktg_u = DK.tile([128, 2048], u8, tag="ktg_u")
                    nc.sync.dma_start(
                        out=ktg_u[:].rearrange("p (q d) -> p q d", d=64),
                        in_=kv_out[t][:, :, 0:64].rearrange(
                            "c (b p) d -> p (c b) d", b=4))
                    vtg_u = DK.tile([128, 2048], u8, tag="vtg_u")
                    nc.sync.dma_start(
                        out=vtg_u[:].rearrange("p (q d) -> p q d", d=64),
                        in_=kv_out[t][:, :, 64:128].rearrange(
                            "c (b p) d -> p (c b) d", b=4))
                    ktg = DK.tile([128, 2048], b16, tag="ktg")
                    nc.vector.tensor_copy(ktg[:], ktg_u[:])
                    vtg = DK.tile([128, 2048], b16, tag="vtg")
                    nc.vector.tensor_copy(vtg[:], vtg_u[:])
                    gbs = [DK.tile([128, 256], b16, tag=f"gb{c}",
                                   name=f"gb{c}_{t}")
                           for c in range(NC8)]
                    nc.vector.memset(gbs[0][0:64, :], 0.0)
                    nc.vector.memset(gbs[NC8 - 1][64:128, :], 0.0)
                    grun2 = DK.tile([128, 256], f32, tag="grun2")
                    nc.vector.memset(grun2[:], 0.0)
                    # whole-t k~ tiles (one ACT per direction)
                    ktf = DK.tile([128, 2048], b16, tag="ktf", bufs=1)
                    nc.scalar.activation(ktf[:], ktg[:], ACT.Copy,
                                         scale=gv2_sb[:, 0:1])
                    ktb = DK.tile([128, 2048], b16, tag="ktb", bufs=1)
                    nc.scalar.activation(ktb[:], ktg[:], ACT.Copy,
                                         scale=gv2_sb[:, 1:2])
                    # merged fwd/bwd recurrence: iteration i handles fwd
                    # chunk i (partitions 0:64) and bwd chunk 7-i (64:128)
                    for i in range(NC8):
                        cF, cB = i, NC8 - 1 - i
                        lp = PSL.tile([128, 256], f32, tag="lp")
                        for b in range(B):
                            nc.tensor.matmul(
                                lp[0:64, b * 64:(b + 1) * 64],
                                ktf[:, cF * 256 + b * 64:
                                    cF * 256 + (b + 1) * 64],
                                vtg[:, (cF * 4 + b) * 64:
                                    (cF * 4 + b + 1) * 64],
                                start=True, stop=True)
                        for b in range(B):
                            nc.tensor.matmul(
                                lp[64:128, b * 64:(b + 1) * 64],
                                ktb[:, cB * 256 + b * 64:
                                    cB * 256 + (b + 1) * 64],
                                vtg[:, (cB * 4 + b) * 64:
                                    (cB * 4 + b + 1) * 64],
                                start=True, stop=True)
                        if cF > 0:
                            nc.scalar.copy(gbs[cF][0:64, :], grun2[0:64, :])
                        if cB < NC8 - 1:
                            nc.scalar.copy(gbs[cB][64:128, :],
                                           grun2[64:128, :])
                        nc.vector.scalar_tensor_tensor(
                            grun2[:], grun2[:], g128_sb[:, 0:1], lp[:],
                            ALU.mult, ALU.add)
                    # --- D3: intra + cross, retention LIF ---
                    ks = DK.tile([64, 4096], b16, tag="ks", bufs=1)
                    nc.sync.dma_start(
                        out=ks[:],
                        in_=qk_sp[64:128, t * 4096:(t + 1) * 4096])
                    qst = DK.tile([128, 4096], b16, tag="qst", bufs=1)
                    nc.sync.dma_start(
                        out=qst[0:64, :],
                        in_=qk_sp[0:64, t * 4096:(t + 1) * 4096])
                    nc.sync.dma_start(
                        out=qst[64:128, :],
                        in_=qk_sp[0:64, t * 4096:(t + 1) * 4096])
                    qt2 = DK.tile([128, 4096], b16, tag="qt2", bufs=1)
                    nc.vector.tensor_mul(
                        qt2[:].rearrange("p (x n) -> p x n", n=128),
                        qst[:].rearrange("p (x n) -> p x n", n=128),
                        gq2_sb[:].unsqueeze(1).broadcast_to([128, 32, 128]))
                    for cp in range(4):
                        ops = []
                        for j in range(2):
                            c = 2 * cp + j
                            atp = PSAT.tile([128, 512], f32, tag="atp")
                            for b in range(B):
                                qof = (4 * t + b) * 1024 + c * 128
                                nc.tensor.matmul(
                                    atp[:, b * 128:(b + 1) * 128],
                                    ks[:, b * 1024 + c * 128:
                                       b * 1024 + (c + 1) * 128],
                                    qk_sp[0:64, qof:qof + 128],
                                    start=True, stop=True)
                            ats = DK.tile([128, 512], b16, tag="ats")
                            nc.vector.tensor_mul(
                                ats[:].rearrange("p (b n) -> p b n", b=4),
                                atp[:].rearrange("p (b n) -> p b n", b=4),
                                dl_sb[:].unsqueeze(1).broadcast_to(
                                    [128, 4, 128]))
                            op = PSO2.tile([64, 512], f32, tag="op")
                            for b in range(B):
                                vof = (c * 4 + b) * 64
                                nc.tensor.matmul(
                                    op[:, b * 128:(b + 1) * 128],
                                    vtg[:, vof:vof + 64],
                                    ats[:, b * 128:(b + 1) * 128],
                                    start=(b == 0), stop=False)
                                nc.tensor.matmul(
                                    op[:, b * 128:(b + 1) * 128],
                                    gbs[c][:, b * 64:(b + 1) * 64],
                                    qt2[:, b * 1024 + c * 128:
                                        b * 1024 + (c + 1) * 128],
                                    start=False, stop=(b == B - 1))
                            ops.append(op)
                        # retention LIF over the chunk pair
                        sc = float(2.0 ** (t - 1))
                        th = float((2.0 ** t) * 0.5)
                        stv = rv_st[:].rearrange("p (b n) -> p b n", b=4)[
                            :, :, cp * 256:(cp + 1) * 256]
                        if t == 0:
                            # ACT writes straight into the (strided) state view
                            for j in range(2):
                                c = 2 * cp + j
                                stc = rv_st[:].rearrange(
                                    "p (b n) -> p b n", b=4)[
                                    :, :, c * 128:(c + 1) * 128]
                                nc.scalar.activation(
                                    stc,
                                    ops[j][:].rearrange(
                                        "p (b n) -> p b n", b=4),
                                    ACT.Copy, scale=sc)
                        else:
                            pa = SP1.tile([64, 1024], b16, tag="ret_tmp")
                            pav = pa[:].rearrange("p (b c n) -> p b c n",
                                                  b=4, c=2)
                            for j in range(2):
                                nc.scalar.activation(
                                    pav[:, :, j:j + 1, :].squeeze(2),
                                    ops[j][:].rearrange(
                                        "p (b n) -> p b n", b=4),
                                    ACT.Copy, scale=sc)
                            nc.vector.tensor_add(
                                stv, stv, pa[:].rearrange(
                                    "p (b n) -> p b n", b=4))
                        rs = SP1.tile([64, 1024], u8, tag="r_spike")
                        rsv = rs[:].rearrange("p (b n) -> p b n", b=4)
                        nc.vector.tensor_single_scalar(rsv, stv, th, ALU.is_ge)
                        if t < T - 1:
                            nc.vector.scalar_tensor_tensor(
                                stv, stv, th, stv, ALU.is_lt, ALU.mult)
                        for j in range(2):
                            nc.sync.dma_start(
                                out=rc_in[t][2 * cp + j, :, :].rearrange(
                                    "p (b n) -> p b n", b=4),
                                in_=rs[:].rearrange(
                                    "p (b c n) -> p b c n", b=4,
                                    c=2)[:, :, j:j + 1, :].squeeze(2))
                    nc.gpsimd.collective_compute(
                        "AllToAll", ALU.bypass, replica_groups=rg,
                        ins=[rc_in[t][:]], outs=[rc_out[t][:]])

        # ================= phase 2: out-proj + MLP (row-shard) =================
        with (
            tc.tile_pool(name="ph2", bufs=1) as P2,
            tc.tile_pool(name="spike2", bufs=3) as SP2,
            tc.tile_pool(name="work", bufs=3) as WP,
        ):
            SPH["p"] = SP2
            w2_sb = P2.tile([128, 16 * C], b16, tag="w2_sb")
            for kk in range(16):
                nc.gpsimd.dma_start(out=w2_sb[:, kk * C:(kk + 1) * C],
                                    in_=w2[kk * 128:(kk + 1) * 128, :])
            w1_sb = P2.tile([128, 4 * HID], b16, tag="w1_sb")
            for kk in range(4):
                nc.gpsimd.dma_start(out=w1_sb[:, kk * HID:(kk + 1) * HID],
                                    in_=w1[kk * 128:(kk + 1) * 128, :])
            p_st = P2.tile([128, 4 * 512], b16, tag="p_st")
            h_st = P2.tile([128, 16 * 512], b16, tag="h_st")
            m_st = P2.tile([128, 4 * 512], b16, tag="m_st")
            x1 = P2.tile([128, 4 * RR], b16, tag="x1")

            with tc.tile_pool(name="psh", bufs=2, space="PSUM") as PSH:
                # ----- stage F: output projection + residual 1 -----
                for fc in range(4):          # fc == t (512-row slabs)
                    rts = []
                    for kk in range(4):
                        rtu = WP.tile([128, 512], mybir.dt.uint8,
                                      tag="wp_rhs_u", bufs=4)
                        nc.scalar.dma_start(out=rtu[0:64, :],
                                            in_=rc_out[fc][2 * kk, :, :])
                        nc.scalar.dma_start(out=rtu[64:128, :],
                                            in_=rc_out[fc][2 * kk + 1, :, :])
                        rt = WP.tile([128, 512], b16, tag="wp_rhs", bufs=8)
                        nc.vector.tensor_copy(rt[:], rtu[:])
                        rts.append(rt)
                    for cp in range(2):      # cc pairs
                        ph2t = PSH.tile([128, 1024], f32, tag="psh")
                        for j in range(2):
                            cc = 2 * cp + j
                            for kk in range(4):
                                nc.tensor.matmul(
                                    ph2t[:, j * 512:(j + 1) * 512],
                                    wp_sb[:, kk * C + cc * 128:
                                          kk * C + (cc + 1) * 128],
                                    rts[kk][:], start=(kk == 0), stop=(kk == 3))
                        sp2 = SP2.tile([128, 1024], b16, tag="p_spike")
                        cc0 = 2 * cp
                        lif_pair(
                            [ph2t[:, 0:512], ph2t[:, 512:1024]],
                            [bp_sb[:, cc0 * T + fc:cc0 * T + fc + 1],
                             bp_sb[:, (cc0 + 1) * T + fc:(cc0 + 1) * T + fc + 1]],
                            p_st[:, cc0 * 512:(cc0 + 2) * 512], sp2[:], fc, 1.0)
                        nc.sync.dma_start(
                            out=ao_d[cc0 * 128:(cc0 + 2) * 128,
                                     fc * 512:(fc + 1) * 512].rearrange(
                                "(j p) c -> p j c", j=2),
                            in_=sp2[:].rearrange("p (j c) -> p j c", j=2))
                        nc.vector.tensor_add(
                            x1[:].rearrange("p (cc r) -> p cc r", cc=4)[
                                :, cc0:cc0 + 2, fc * 512:(fc + 1) * 512],
                            sp2[:].rearrange("p (j c) -> p j c", j=2),
                            xrT_sb[:].rearrange("p (cc r) -> p cc r", cc=4)[
                                :, cc0:cc0 + 2, fc * 512:(fc + 1) * 512])

                # ----- stage H: MLP + residual 2 -----
                with tc.tile_pool(name="psm", bufs=1, space="PSUM") as PSM:
                    for rq in range(4):      # rq == t
                        pm = [PSM.tile([128, 1024], f32, tag=f"pm{cp}",
                                       name=f"pm{cp}_{rq}") for cp in range(2)]
                        for hp in range(8):          # hc pairs
                            ph = PSH.tile([128, 1024], f32, tag="psh")
                            for j in range(2):
                                hc = 2 * hp + j
                                for kk in range(4):
                                    nc.tensor.matmul(
                                        ph[:, j * 512:(j + 1) * 512],
                                        w1_sb[:, kk * HID + hc * 128:
                                              kk * HID + (hc + 1) * 128],
                                        x1[:, kk * RR + rq * 512:
                                           kk * RR + (rq + 1) * 512],
                                        start=(kk == 0), stop=(kk == 3))
                            hs = SP2.tile([128, 1024], b16, tag="h_spike")
                            hc0 = 2 * hp
                            lif_pair(
                                [ph[:, 0:512], ph[:, 512:1024]],
                                [b1_sb[:, hc0 * T + rq:hc0 * T + rq + 1],
                                 b1_sb[:, (hc0 + 1) * T + rq:
                                       (hc0 + 1) * T + rq + 1]],
                                h_st[:, hc0 * 512:(hc0 + 2) * 512], hs[:],
                                rq, 1.0)
                            for j in range(2):
                                hc = 2 * hp + j
                                for cc in range(4):
                                    nc.tensor.matmul(
                                        pm[cc // 2][:, (cc % 2) * 512:
                                                    (cc % 2) * 512 + 512],
                                        w2_sb[:, hc * C + cc * 128:
                                              hc * C + (cc + 1) * 128],
                                        hs[:, j * 512:(j + 1) * 512],
                                        start=(hc == 0), stop=(hc == 15))
                        for cp in range(2):
                            cc0 = 2 * cp
                            ms2 = SP2.tile([128, 1024], b16, tag="m_spike")
                            lif_pair(
                                [pm[cp][:, 0:512], pm[cp][:, 512:1024]],
                                [b2_sb[:, cc0 * T + rq:cc0 * T + rq + 1],
                                 b2_sb[:, (cc0 + 1) * T + rq:
                                       (cc0 + 1) * T + rq + 1]],
                                m_st[:, cc0 * 512:(cc0 + 2) * 512], ms2[:],
                                rq, 1.0)
                            xf = WP.tile([128, 1024], f32, tag="xf")
                            nc.sync.dma_start(
                                out=xf[:].rearrange("p (j c) -> p j c", j=2),
                                in_=xrf[cc0 * 128:(cc0 + 2) * 128,
                                        rq * 512:(rq + 1) * 512].rearrange(
                                    "(j p) c -> p j c", j=2))
                            av = WP.tile([128, 1024], b16, tag="av")
                            nc.sync.dma_start(
                                out=av[:].rearrange("p (j c) -> p j c", j=2),
                                in_=ao_d[cc0 * 128:(cc0 + 2) * 128,
                                         rq * 512:(rq + 1) * 512].rearrange(
                                    "(j p) c -> p j c", j=2))
                            ot = WP.tile([128, 1024], f32, tag="ot")
                            nc.vector.tensor_add(ot[:], xf[:], av[:])
                            nc.vector.tensor_add(ot[:], ot[:], ms2[:])
                            nc.sync.dma_start(
                                out=out_e[cc0 * 128:(cc0 + 2) * 128,
                                          rq * 512:(rq + 1) * 512].rearrange(
                                    "(j p) c -> p j c", j=2),
                                in_=ot[:].rearrange("p (j c) -> p j c", j=2))

    nc.compile()
    return nc


def _host_prep(inputs):
    x = np.asarray(inputs["x"], np.float32)          # (T,B,N,C)
    xT_b = x.transpose(3, 0, 1, 2).reshape(C, RF).astype(bf16)
    xT3_b = xT_b.reshape(4, 128, RF).transpose(1, 0, 2).copy()

    Wq, bq_ = _fold_bn(inputs["Wq"], inputs["bq"], inputs["gq"],
                       inputs["betaq"], inputs["rmq"], inputs["rvq"])
    Wk, bk_ = _fold_bn(inputs["Wk"], inputs["bk"], inputs["gk"],
                       inputs["betak"], inputs["rmk"], inputs["rvk"])
    Wv, bv_ = _fold_bn(inputs["Wv"], inputs["bv"], inputs["gv"],
                       inputs["betav"], inputs["rmv"], inputs["rvv"])
    Wp, bp_ = _fold_bn(inputs["Wp"], inputs["bp"], inputs["gp"],
                       inputs["betap"], inputs["rmp"], inputs["rvp"])
    W1, b1_ = _fold_bn(inputs["W1"], inputs["b1"], inputs["g1"],
                       inputs["beta1"], inputs["rm1"], inputs["rv1"])
    W2, b2_ = _fold_bn(inputs["W2"], inputs["b2"], inputs["g2"],
                       inputs["beta2"], inputs["rm2"], inputs["rv2"])

    tsc = np.array([2.0 ** (t - 1) for t in range(T)], np.float32)

    def pack_bias(bvec, nchunk):
        out = np.zeros((128, nchunk * T), np.float32)
        for ch in range(nchunk):
            for t in range(T):
                out[:, ch * T + t] = tsc[t] * bvec[ch * 128:(ch + 1) * 128]
        return out

    # k|v interleave: feature block hh*128 + (0:64 -> k head hh, 64:128 -> v)
    wkv_m = np.zeros((C, 2 * C), np.float32)
    bkv_m = np.zeros((1, 2 * C), np.float32)
    for hh in range(H):
        wkv_m[:, hh * 128:hh * 128 + 64] = Wk[:, hh * 64:(hh + 1) * 64]
        wkv_m[:, hh * 128 + 64:hh * 128 + 128] = Wv[:, hh * 64:(hh + 1) * 64]
        bkv_m[0, hh * 128:hh * 128 + 64] = bk_[hh * 64:(hh + 1) * 64]
        bkv_m[0, hh * 128 + 64:hh * 128 + 128] = bv_[hh * 64:(hh + 1) * 64]

    gamma = (1.0 - 2.0 ** (-5.0 - np.arange(H, dtype=np.float64)))
    idx = np.arange(128, dtype=np.float64)

    in_maps = []
    for cid in range(NCORES):
        h = cid
        g = gamma[h]
        dl_m = (SCALE * g ** np.abs(idx[:, None] - idx[None, :])).astype(
            np.float32)
        gq2_m = np.zeros((128, 128), np.float32)
        gq2_m[0:64, :] = SCALE * g ** (idx + 1.0)[None, :]
        gq2_m[64:128, :] = SCALE * g ** (127.0 - idx)[None, :]
        gv2_m = np.zeros((128, 2), np.float32)
        gv2_m[:, 0] = g ** (127.0 - idx)
        gv2_m[:, 1] = g ** (idx + 1.0)
        g128_m = np.full((128, 1), g ** 128.0, np.float32)

        xs = x[:, :, 128 * cid:128 * (cid + 1), :]       # (T,B,128,C)
        xrT_f = xs.transpose(3, 0, 1, 2).reshape(C, RR)

        wqk_m = np.concatenate([Wq[:, h * D:(h + 1) * D],
                                Wk[:, h * D:(h + 1) * D]], axis=1)
        bqk_m = np.zeros((128, T), np.float32)
        for t in range(T):
            bqk_m[0:64, t] = tsc[t] * bq_[h * D:(h + 1) * D]
            bqk_m[64:128, t] = tsc[t] * bk_[h * D:(h + 1) * D]

        in_maps.append({
            "xT3": xT3_b,
            "xrT": xrT_f.astype(bf16),
            "xrf": xrT_f.astype(np.float32),
            "wqk": wqk_m.astype(bf16),
            "bqk": bqk_m,
            "wkv": wkv_m.astype(bf16),
            "bkv": bkv_m.astype(bf16),
            "wp": Wp.astype(bf16),
            "bp": pack_bias(bp_, 4),
            "w1": W1.astype(bf16),
            "b1": pack_bias(b1_, 16),
            "w2": W2.astype(bf16),
            "b2": pack_bias(b2_, 4),
            "dl": dl_m.astype(bf16),
            "gq2": gq2_m.astype(bf16),
            "gv2": gv2_m,
            "g128": g128_m,
        })
    return in_maps


def _install_trace_hook():
    import types
    import antenv
    if "antenv.axon_hooks" in sys.modules:
        return True
    mod = types.ModuleType("antenv.axon_hooks")
    _h = [None]
    mod.set_axon_ntff_profile_hook = lambda hk: _h.__setitem__(0, hk)
    mod.get_axon_ntff_profile_hook = lambda: _h[0]
    sys.modules["antenv.axon_hooks"] = mod
    antenv.axon_hooks = mod
    try:
        from trn_agent_boot.trn_boot import _ntff_profile_via_ctypes
        hook = _ntff_profile_via_ctypes("/opt/axon/libaxon_pjrt.so")
        mod.set_axon_ntff_profile_hook(hook)
        return hook is not None
    except Exception:
        return False


def kernel(**inputs):
    global LAST_EXEC_NS, _CACHED
    from concourse.bass_utils import run_bass_kernel_spmd

    trace = os.environ.get("BASS_KERNEL_TRACE", "0") == "1"
    if trace:
        _install_trace_hook()

    if _CACHED is None:
        _CACHED = _build_nc()
    nc = _CACHED

    in_maps = _host_prep(inputs)
    res = run_bass_kernel_spmd(nc, in_maps, core_ids=list(range(NCORES)),
                               trace=trace)
    LAST_EXEC_NS = res.exec_time_ns

    full = np.empty((T, B, N, C), np.float32)
    for cid in range(NCORES):
        oc = res.results[cid]["out"]                    # (C, RR) f32
        full[:, :, 128 * cid:128 * (cid + 1), :] = (
            oc.reshape(C, T, B, NL).transpose(1, 2, 3, 0))
    return full
